# revision 43
# baseline (speedup 1.0000x reference)
"""Multi-head attention (B=2, G=2, QLEN=KVLEN=1024, DIN=1024, H=16) on 8 TRN2
NeuronCores, pure data-parallel: core c handles (bg = c//2, q-half = c%2).

Per-core dataflow (projection/score matmuls in float32r — 11-bit mantissa at
full PE speed, fp32 PSUM accumulation; host pre-rounds inputs to the f32r
grid; the attention-value path runs in bf16):

  qT  [qk,  q ] = WqT.T @ xqT   (+bq per-partition on eviction)       f32r
  kT  [qk,  kv] = WkT.T @ xkvT  (+bk)                                 f32r
  v   [kv,  vc] = xkvT.T @ WvT  (65th col per head = ones -> sums)    bf16
  per head pair (scores of the two heads row-packed on the PE array):
    scoresT[kv, q] = kT_h.T @ qT_h          (K=64, f32r)
    expT = exp(0.125 * scoresT)             (ACT, bf16 out)
    r[65, q] = v_aug_h.T @ expT             (bf16 x bf16, row 64 = sums)
    bcast[128, q] = ones.T @ sums           (K=1 matmul partition-broadcast)
    recip = 1/bcast                         (DVE)
    probsT = expT * recip -> DMA out        (f32)
    rT_h = r[0:64] * recip -> resultT       (f32r)
  outT [oc, q] = WoT.T @ rT (+ (bo + Wo@bv) per-partition on eviction)

Heads 0-7 are interleaved with the second half of the V projection so the
probs output DMA starts as early as possible (DMA is the roofline).
Outputs per core: probsT [16, 8, 128, 512] and outT [8, 128, 512]; the host
reassembles the full (out, attn_probs) tuple.
"""
import numpy as np

import concourse.bass as bass
import concourse.mybir as mybir
import concourse.tile as tile
from concourse.tile import add_dep_helper
from concourse import bacc
from concourse.bass_utils import run_bass_kernel_spmd

B, G, QLEN, KVLEN = 2, 2, 1024, 1024
DIN = 1024
H = 16
QK, VC, OC = 1024, 1024, 1024
P = 128
QS = QLEN // 2          # q rows per core
KD = DIN // P           # contraction chunks
VH = VC // H            # 64 v-channels per head
NCORES = 8

F32R = mybir.dt.float32r
F32 = mybir.dt.float32
BF16 = mybir.dt.bfloat16
Exp = mybir.ActivationFunctionType.Exp
Identity = mybir.ActivationFunctionType.Identity

_NC_CACHE_R = {}


def round_f32r(x: np.ndarray) -> np.ndarray:
    """Round fp32 to the fp32r grid (1-8-11, RNE) — what the PE consumes."""
    u = np.ascontiguousarray(x, dtype=np.float32).view(np.uint32).copy()
    u += 0x7FF + ((u >> 12) & np.uint32(1))
    u &= np.uint32(0xFFFFF000)
    return u.view(np.float32)


def _build(repeat: int = 1):
    nc = bacc.Bacc(None, target_bir_lowering=False, debug=False)

    xqT_d = nc.declare_dram_parameter("xqT", [KD, P, QS], F32R, isOutput=False)
    xkvT_d = nc.declare_dram_parameter("xkvT", [KD, P, KVLEN], F32R, isOutput=False)
    WqT_d = nc.declare_dram_parameter("WqT", [KD, P, QK], F32R, isOutput=False)
    WkT_d = nc.declare_dram_parameter("WkT", [KD, P, QK], F32R, isOutput=False)
    WvT_d = nc.declare_dram_parameter("WvT", [KD, P, VC], F32R, isOutput=False)
    WoT_d = nc.declare_dram_parameter("WoT", [VC // P, P, OC], BF16, isOutput=False)
    bias_d = nc.declare_dram_parameter("bias_all", [P, 24], F32, isOutput=False)
    probsT_d = nc.declare_dram_parameter(
        "probsT", [H, KVLEN // P, P, QS], BF16, isOutput=True
    )
    outT_d = nc.declare_dram_parameter("outT", [OC // P, P, QS], BF16, isOutput=True)

    with tile.TileContext(nc) as tc:
        with (
            tc.tile_pool(name="const", bufs=1) as constp,
            tc.tile_pool(name="persist", bufs=1) as persist,
            tc.tile_pool(name="wpool", bufs=8) as wpool,
        ):
            # constants
            bias_sb = constp.tile([P, 24], F32, name="bias_sb")
            bq_sb = bias_sb[:, 0:8]
            bk_sb = bias_sb[:, 8:16]
            bco_sb = bias_sb[:, 16:24]
            onesf = constp.tile([P, H], F32, name="onesf")
            nc.any.memset(onesf[:], 1.0)
            ones1_r = constp.tile([1, P], F32R, name="ones1_r")
            onesf_row = constp.tile([1, P], F32, name="onesf_row")
            nc.any.memset(onesf_row[:], 1.0)
            nc.vector.tensor_copy(ones1_r[:], onesf_row[:])
            ones_col_bf = constp.tile([P, 1], BF16, name="ones_col_bf")
            nc.vector.tensor_copy(ones_col_bf[:], onesf[:, 0:1])

            # repeats share pool slots -> they serialize; repeat>1 is only
            # for exec-time calibration
            for _rep in range(repeat):
                qT_sb = persist.tile(
                    [P, QK // P, QS], F32R, tag="qT", name="qT_sb"
                )
                kT_sb = persist.tile(
                    [P, QK // P, KVLEN], F32R, tag="kT", name="kT_sb"
                )
                v_sb = persist.tile(
                    [P, KVLEN // P, H, VH + 1], BF16, tag="v", name="v_sb"
                )
                rT_sb = persist.tile(
                    [P, VC // P, QS], BF16, tag="rT", name="rT_sb"
                )
                for k in range(KVLEN // P):
                    nc.vector.tensor_copy(v_sb[:, k, :, VH], onesf[:])

                # ---- Q projection (own SBUF/PSUM scope, freed afterwards)
                with (
                    tc.tile_pool(name="xqp", bufs=1) as xqp,
                    tc.tile_pool(name="qpp", bufs=4, space="PSUM") as qpp,
                ):
                    xqT_sb = xqp.tile([P, KD, QS], F32R, name="xqT_sb")
                    wq = []
                    for k in range(KD):
                        nc.sync.dma_start(out=xqT_sb[:, k, :], in_=xqT_d[k])
                        wt = wpool.tile([P, 1024], F32R, tag="w", name="wt")
                        nc.sync.dma_start(out=wt[:], in_=WqT_d[k])
                        wq.append(wt)
                    nc.sync.dma_start(out=bias_sb[:], in_=bias_d[:])
                    for mh in range(2):
                        qps = [
                            qpp.tile([P, QS], F32, tag="qpp", name="qps")
                            for _ in range(4)
                        ]
                        for k in range(KD):
                            for mi in range(4):
                                m = mh * 4 + mi
                                nc.tensor.matmul(
                                    qps[mi][:],
                                    wq[k][:, m * P:(m + 1) * P],
                                    xqT_sb[:, k, :],
                                    start=(k == 0),
                                    stop=(k == KD - 1),
                                )
                        for mi in range(4):
                            m = mh * 4 + mi
                            nc.vector.tensor_scalar_add(
                                qT_sb[:, m, :], qps[mi][:], bq_sb[:, m:m + 1]
                            )

                # ---- K/V projections + attention heads (interleaved)
                with (
                    tc.tile_pool(name="xkvp", bufs=1) as xkvp,
                    tc.tile_pool(name="eTp", bufs=6) as eTp,
                    tc.tile_pool(name="pTp", bufs=4) as pTp,
                    tc.tile_pool(name="bcrp", bufs=2) as bcrp,
                    tc.tile_pool(name="sump", bufs=2) as sump,
                    tc.tile_pool(name="brp", bufs=1) as brp,
                    tc.tile_pool(name="pp", bufs=2, space="PSUM") as pp,
                    tc.tile_pool(name="psc", bufs=2, space="PSUM") as psc,
                    tc.tile_pool(name="pres", bufs=2, space="PSUM") as pres,
                    tc.tile_pool(name="pbc", bufs=2, space="PSUM") as pbc,
                ):
                    xkvT_sb = xkvp.tile([P, KD, KVLEN], F32R, name="xkvT_sb")
                    wk = []
                    for k in range(KD):
                        nc.sync.dma_start(out=xkvT_sb[:, k, :], in_=xkvT_d[k])
                        wt = wpool.tile([P, 1024], F32R, tag="w", name="wt")
                        nc.sync.dma_start(out=wt[:], in_=WkT_d[k])
                        wk.append(wt)

                    # K projection: k-outer (kv half, m quarter) passes,
                    # only 2 PSUM banks held
                    for n in range(2):
                        for mh in range(4):
                            kps = [
                                pp.tile([P, QS], F32, tag="pp", name="kps")
                                for _ in range(2)
                            ]
                            for k in range(KD):
                                for mi in range(2):
                                    m = mh * 2 + mi
                                    nc.tensor.matmul(
                                        kps[mi][:],
                                        wk[k][:, m * P:(m + 1) * P],
                                        xkvT_sb[:, k, n * QS:(n + 1) * QS],
                                        start=(k == 0),
                                        stop=(k == KD - 1),
                                    )
                            for mi in range(2):
                                m = mh * 2 + mi
                                nc.vector.tensor_scalar_add(
                                    kT_sb[:, m, n * QS:(n + 1) * QS],
                                    kps[mi][:], bk_sb[:, m:m + 1]
                                )

                    wv = []
                    for k in range(KD):
                        wt = wpool.tile([P, 1024], F32R, tag="w", name="wt")
                        nc.sync.dma_start(out=wt[:], in_=WvT_d[k])
                        wv.append(wt)

                    def v_quarter(n, mq4):
                        """One (vc-half, m-quarter) k-outer V-projection pass
                        holding only 2 PSUM banks."""
                        vps = [
                            pp.tile([P, QS], F32, tag="pp", name="vps")
                            for _ in range(2)
                        ]
                        for k in range(KD):
                            for mi in range(2):
                                m = mq4 * 2 + mi
                                nc.tensor.matmul(
                                    vps[mi][:],
                                    xkvT_sb[:, k, m * P:(m + 1) * P],
                                    wv[k][:, n * QS:(n + 1) * QS],
                                    start=(k == 0),
                                    stop=(k == KD - 1),
                                )
                        for mi in range(2):
                            m = mq4 * 2 + mi
                            nc.scalar.copy(
                                v_sb[:, m, 8 * n:8 * n + 8, 0:VH],
                                vps[mi][:].rearrange("p (h c) -> p h c", h=8),
                            )

                    def probs_part(h, eT):
                        """scores/exp already done; softmax sums via a
                        ones-column matmul (no V dependency), normalize and
                        DMA the probs out."""
                        bc = pbc.tile([P, QS], F32, tag="bc", name="bc")
                        for k in range(KVLEN // P):
                            nc.tensor.matmul(
                                bc[0:1, :],
                                ones_col_bf[:],
                                eT[:, k, :],
                                start=(k == 0),
                                stop=(k == KVLEN // P - 1),
                            )
                        srt = sump.tile([1, QS], F32R, tag="sums", name="srt")
                        nc.vector.tensor_copy(srt[:], bc[0:1, :])
                        nc.tensor.matmul(
                            bc[:], ones1_r[:], srt[:], start=True, stop=True
                        )
                        bcr = bcrp.tile([P, QS], F32, tag="bcr", name="bcr")
                        nc.vector.reciprocal(bcr[:], bc[:])
                        for jg in range(4):
                            pT = pTp.tile([P, 2, QS], BF16, tag="pT", name="pT")
                            for jj in range(2):
                                j = jg * 2 + jj
                                eng = nc.vector if jj == 0 else nc.gpsimd
                                eng.tensor_mul(
                                    pT[:, jj, :], eT[:, j, :], bcr[:]
                                )
                            nc.sync.dma_start(
                                out=probsT_d[h, jg * 2:(jg + 1) * 2].rearrange(
                                    "j p q -> p j q"
                                ),
                                in_=pT[:],
                            )

                    def result_part(h, eT):
                        """attention result for one head; the reciprocal is
                        recomputed from the v-aug sums row so nothing from
                        probs_part needs to stay alive."""
                        mq = h >> 1
                        bp = 64 * (h & 1)
                        rp = pres.tile([VH + 1, QS], F32, tag="res", name="rp")
                        for k in range(KVLEN // P):
                            nc.tensor.matmul(
                                rp[:],
                                v_sb[:, k, h, :],
                                eT[:, k, :],
                                start=(k == 0),
                                stop=(k == KVLEN // P - 1),
                            )
                        srt = sump.tile([1, QS], F32R, tag="sums", name="srt")
                        nc.vector.tensor_copy(srt[:], rp[VH:VH + 1, :])
                        bc2 = pbc.tile([P, QS], F32, tag="bc", name="bc2")
                        nc.tensor.matmul(
                            bc2[0:VH, :], ones1_r[:, 0:VH], srt[:],
                            start=True, stop=True,
                        )
                        br = brp.tile([VH, QS], F32, tag="br", name="br")
                        nc.vector.reciprocal(br[:], bc2[0:VH, :])
                        return nc.vector.tensor_mul(
                            rT_sb[bp:bp + 64, mq, :], rp[0:VH, :], br[:]
                        )

                    def full_head(h, eT):
                        """Fused head: result matmul first (v ready), softmax
                        sums from the v-aug row, then probs + resultT."""
                        mq = h >> 1
                        bp = 64 * (h & 1)
                        rp = pres.tile([VH + 1, QS], F32, tag="res", name="rp")
                        for k in range(KVLEN // P):
                            nc.tensor.matmul(
                                rp[:],
                                v_sb[:, k, h, :],
                                eT[:, k, :],
                                start=(k == 0),
                                stop=(k == KVLEN // P - 1),
                            )
                        srt = sump.tile([1, QS], F32R, tag="sums", name="srt")
                        nc.vector.tensor_copy(srt[:], rp[VH:VH + 1, :])
                        bc = pbc.tile([P, QS], F32, tag="bc", name="bc")
                        nc.tensor.matmul(
                            bc[:], ones1_r[:], srt[:], start=True, stop=True
                        )
                        bcr = bcrp.tile([P, QS], F32, tag="bcr", name="bcr")
                        nc.vector.reciprocal(bcr[:], bc[:])
                        for jg in range(4):
                            pT = pTp.tile([P, 2, QS], BF16, tag="pT", name="pT")
                            for jj in range(2):
                                j = jg * 2 + jj
                                eng = nc.vector if jj == 0 else nc.gpsimd
                                eng.tensor_mul(
                                    pT[:, jj, :], eT[:, j, :], bcr[:]
                                )
                            nc.sync.dma_start(
                                out=probsT_d[h, jg * 2:(jg + 1) * 2].rearrange(
                                    "j p q -> p j q"
                                ),
                                in_=pT[:],
                            )
                        nc.vector.tensor_mul(
                            rT_sb[bp:bp + 64, mq, :], rp[0:VH, :], bcr[0:64, :]
                        )

                    def pair_scores(i, after=None):
                        """scores+exp for heads (2i, 2i+1), j-interleaved so
                        the K=64 row-packed pairs overlap on the PE array.
                        `after` pins an ordering edge so the scheduler keeps
                        this behind the result whose expT slots we reuse."""
                        h0, h1 = 2 * i, 2 * i + 1
                        mq = i
                        eT0 = eTp.tile(
                            [P, KVLEN // P, QS], BF16, tag="eT", name="eT0"
                        )
                        eT1 = eTp.tile(
                            [P, KVLEN // P, QS], BF16, tag="eT", name="eT1"
                        )
                        eTs = (eT0, eT1)
                        for j in range(KVLEN // P):
                            for x, hh in enumerate((h0, h1)):
                                bp = 64 * (hh & 1)
                                sc = psc.tile(
                                    [P, QS], F32, tag="sc", name="sc"
                                )
                                mm = nc.tensor.matmul(
                                    sc[:],
                                    kT_sb[bp:bp + 64, mq, j * P:(j + 1) * P],
                                    qT_sb[bp:bp + 64, mq, :],
                                    start=True,
                                    stop=True,
                                )
                                if after is not None:
                                    add_dep_helper(
                                        mm.ins, after.ins, sync=False,
                                        reason="expT slot pipeline order",
                                    )
                                    after = None
                                nc.scalar.activation(
                                    eTs[x][:, j, :], sc[:], Exp, scale=0.125
                                )
                        return eTs

                    # Schedule: pairs 0,1,4,5 run probs-first with
                    # ones-column sums (no V dependency) interleaved with the
                    # V quarter passes; their results follow once the V half
                    # is ready. Pairs 2,3,6,7 run fused (V already there).
                    eTs = {}

                    def S(i):
                        eTs[i] = pair_scores(i)

                    def Pp(i):
                        probs_part(2 * i, eTs[i][0])
                        probs_part(2 * i + 1, eTs[i][1])

                    def R(i):
                        result_part(2 * i, eTs[i][0])
                        result_part(2 * i + 1, eTs[i][1])
                        del eTs[i]

                    def F(i):
                        full_head(2 * i, eTs[i][0])
                        full_head(2 * i + 1, eTs[i][1])
                        del eTs[i]

                    S(0); Pp(0)
                    v_quarter(0, 0)
                    S(1); Pp(1)
                    v_quarter(0, 1); v_quarter(0, 2); v_quarter(0, 3)
                    R(0); S(2); F(2)
                    R(1); S(3); F(3)
                    v_quarter(1, 0); S(4); Pp(4)
                    v_quarter(1, 1); S(5); Pp(5)
                    v_quarter(1, 2); v_quarter(1, 3)
                    wo = []
                    for k in range(VC // P):
                        wt = wpool.tile([P, 1024], BF16, tag="w", name="wt")
                        nc.sync.dma_start(out=wt[:], in_=WoT_d[k])
                        wo.append(wt)
                    R(4); S(6); F(6)
                    R(5); S(7); F(7)

                # ---- output projection
                with (
                    tc.tile_pool(name="otp", bufs=2) as otp,
                    tc.tile_pool(name="ppo", bufs=4, space="PSUM") as ppo,
                ):
                    # k-outer over m-halves: chunks consumed in
                    # head-completion order so most of the projection
                    # overlaps the last heads
                    for mh in range(2):
                        pos = [
                            ppo.tile([P, QS], F32, tag="po", name="po")
                            for _ in range(4)
                        ]
                        for k in range(VC // P):
                            for mi in range(4):
                                m = mh * 4 + mi
                                nc.tensor.matmul(
                                    pos[mi][:],
                                    wo[k][:, m * P:(m + 1) * P],
                                    rT_sb[:, k, :],
                                    start=(k == 0),
                                    stop=(k == VC // P - 1),
                                )
                        for mi in range(4):
                            m = mh * 4 + mi
                            ot = otp.tile([P, QS], BF16, tag="ot", name="ot")
                            nc.vector.tensor_scalar_add(
                                ot[:], pos[mi][:], bco_sb[:, m:m + 1]
                            )
                            nc.sync.dma_start(out=outT_d[m], in_=ot[:])

    nc.compile()
    return nc


def get_nc(repeat: int = 1):
    if repeat not in _NC_CACHE_R:
        _NC_CACHE_R[repeat] = _build(repeat)
    return _NC_CACHE_R[repeat]


def make_in_maps(inputs_q, inputs_kv, Wq, bq, Wk, bk, Wv, bv, Wo, bo):
    inputs_q = np.asarray(inputs_q, dtype=np.float32)
    inputs_kv = np.asarray(inputs_kv, dtype=np.float32)
    Wq = np.asarray(Wq, dtype=np.float32)
    Wk = np.asarray(Wk, dtype=np.float32)
    Wv = np.asarray(Wv, dtype=np.float32)
    Wo = np.asarray(Wo, dtype=np.float32)
    bq = np.asarray(bq, dtype=np.float32)
    bk = np.asarray(bk, dtype=np.float32)
    bv = np.asarray(bv, dtype=np.float32)
    bo = np.asarray(bo, dtype=np.float32)

    WqT = round_f32r(Wq.T).reshape(KD, P, QK)
    WkT = round_f32r(Wk.T).reshape(KD, P, QK)
    WvT = round_f32r(Wv.T).reshape(KD, P, VC)
    import ml_dtypes
    WoT = np.ascontiguousarray(Wo.T).astype(ml_dtypes.bfloat16).reshape(VC // P, P, OC)
    bco = (bo + Wo @ bv).astype(np.float32)
    bias_all = np.concatenate([
        bq.reshape(8, P).T, bk.reshape(8, P).T, bco.reshape(8, P).T
    ], axis=1)  # [P, 24]

    in_maps = []
    for c in range(NCORES):
        bg, half = divmod(c, 2)
        b, g = divmod(bg, G)
        xq = inputs_q[b, g, half * QS:(half + 1) * QS, :]       # [QS, DIN]
        xkv = inputs_kv[b, g]                                   # [KVLEN, DIN]
        xqT = round_f32r(np.ascontiguousarray(xq.T)).reshape(KD, P, QS)
        xkvT = round_f32r(np.ascontiguousarray(xkv.T)).reshape(KD, P, KVLEN)
        in_maps.append({
            "xqT": xqT, "xkvT": xkvT,
            "WqT": WqT, "WkT": WkT, "WvT": WvT, "WoT": WoT,
            "bias_all": bias_all,
        })
    return in_maps


def assemble(results):
    out = np.empty((B, G, QLEN, OC), np.float32)
    probs = np.empty((B, G, H, QLEN, KVLEN), np.float32)
    for c in range(NCORES):
        bg, half = divmod(c, 2)
        b, g = divmod(bg, G)
        qs = slice(half * QS, (half + 1) * QS)
        outT = np.asarray(
            results[c]["outT"], dtype=np.float32
        ).reshape(OC, QS)                                        # [oc, q]
        out[b, g, qs, :] = outT.T
        pT = np.asarray(
            results[c]["probsT"], dtype=np.float32
        ).reshape(H, KVLEN, QS)                                  # [h, kv, q]
        probs[b, g, :, qs, :] = pT.transpose(0, 2, 1)
    return out, probs


def kernel(inputs_q, inputs_kv, Wq, bq, Wk, bk, Wv, bv, Wo, bo):
    nc = get_nc()
    in_maps = make_in_maps(
        inputs_q, inputs_kv, Wq, bq, Wk, bk, Wv, bv, Wo, bo
    )
    res = run_bass_kernel_spmd(nc, in_maps, core_ids=list(range(NCORES)))
    return assemble(res.results)


# revision 46
# speedup vs baseline: 192.7443x; 192.7443x over previous
"""Multi-head attention (B=2, G=2, QLEN=KVLEN=1024, DIN=1024, H=16) on 8 TRN2
NeuronCores, pure data-parallel: core c handles (bg = c//2, q-half = c%2).

Per-core dataflow (projection/score matmuls in float32r — 11-bit mantissa at
full PE speed, fp32 PSUM accumulation; host pre-rounds inputs to the f32r
grid; the attention-value path runs in bf16):

  qT  [qk,  q ] = WqT.T @ xqT   (+bq per-partition on eviction)       f32r
  kT  [qk,  kv] = WkT.T @ xkvT  (+bk)                                 f32r
  v   [kv,  vc] = xkvT.T @ WvT  (65th col per head = ones -> sums)    bf16
  per head pair (scores of the two heads row-packed on the PE array):
    scoresT[kv, q] = kT_h.T @ qT_h          (K=64, f32r)
    expT = exp(0.125 * scoresT)             (ACT, bf16 out)
    r[65, q] = v_aug_h.T @ expT             (bf16 x bf16, row 64 = sums)
    bcast[128, q] = ones.T @ sums           (K=1 matmul partition-broadcast)
    recip = 1/bcast                         (DVE)
    probsT = expT * recip -> DMA out        (f32)
    rT_h = r[0:64] * recip -> resultT       (f32r)
  outT [oc, q] = WoT.T @ rT (+ (bo + Wo@bv) per-partition on eviction)

Heads 0-7 are interleaved with the second half of the V projection so the
probs output DMA starts as early as possible (DMA is the roofline).
Outputs per core: probsT [16, 8, 128, 512] and outT [8, 128, 512]; the host
reassembles the full (out, attn_probs) tuple.
"""
import numpy as np

import concourse.bass as bass
import concourse.mybir as mybir
import concourse.tile as tile
from concourse.tile import add_dep_helper
from concourse import bacc
from concourse.bass_utils import run_bass_kernel_spmd

B, G, QLEN, KVLEN = 2, 2, 1024, 1024
DIN = 1024
H = 16
QK, VC, OC = 1024, 1024, 1024
P = 128
QS = QLEN // 2          # q rows per core
KD = DIN // P           # contraction chunks
VH = VC // H            # 64 v-channels per head
NCORES = 8

F32R = mybir.dt.float32r
F32 = mybir.dt.float32
BF16 = mybir.dt.bfloat16
Exp = mybir.ActivationFunctionType.Exp
Identity = mybir.ActivationFunctionType.Identity

_NC_CACHE_R = {}


def round_f32r(x: np.ndarray) -> np.ndarray:
    """Round fp32 to the fp32r grid (1-8-11, RNE) — what the PE consumes."""
    u = np.ascontiguousarray(x, dtype=np.float32).view(np.uint32).copy()
    u += 0x7FF + ((u >> 12) & np.uint32(1))
    u &= np.uint32(0xFFFFF000)
    return u.view(np.float32)


def _build(repeat: int = 1):
    nc = bacc.Bacc(None, target_bir_lowering=False, debug=False)

    xqT_d = nc.declare_dram_parameter("xqT", [KD, P, QS], F32R, isOutput=False)
    xkvT_d = nc.declare_dram_parameter("xkvT", [KD, P, KVLEN], F32R, isOutput=False)
    WqT_d = nc.declare_dram_parameter("WqT", [KD, P, QK], F32R, isOutput=False)
    WkT_d = nc.declare_dram_parameter("WkT", [KD, P, QK], F32R, isOutput=False)
    WvT_d = nc.declare_dram_parameter("WvT", [KD, P, VC], F32R, isOutput=False)
    WoT_d = nc.declare_dram_parameter("WoT", [VC // P, P, OC], BF16, isOutput=False)
    bias_d = nc.declare_dram_parameter("bias_all", [P, 24], F32, isOutput=False)
    probsT_d = nc.declare_dram_parameter(
        "probsT", [H, KVLEN // P, P, QS], BF16, isOutput=True
    )
    outT_d = nc.declare_dram_parameter("outT", [OC // P, P, QS], BF16, isOutput=True)

    with tile.TileContext(nc) as tc:
        with (
            tc.tile_pool(name="const", bufs=1) as constp,
            tc.tile_pool(name="persist", bufs=1) as persist,
            tc.tile_pool(name="wpool", bufs=8) as wpool,
        ):
            # constants
            bias_sb = constp.tile([P, 24], F32, name="bias_sb")
            bq_sb = bias_sb[:, 0:8]
            bk_sb = bias_sb[:, 8:16]
            bco_sb = bias_sb[:, 16:24]
            onesf = constp.tile([P, H], F32, name="onesf")
            nc.any.memset(onesf[:], 1.0)
            ones1_r = constp.tile([1, P], F32R, name="ones1_r")
            onesf_row = constp.tile([1, P], F32, name="onesf_row")
            nc.any.memset(onesf_row[:], 1.0)
            nc.vector.tensor_copy(ones1_r[:], onesf_row[:])
            ones_col_bf = constp.tile([P, 1], BF16, name="ones_col_bf")
            nc.vector.tensor_copy(ones_col_bf[:], onesf[:, 0:1])

            # repeats share pool slots -> they serialize; repeat>1 is only
            # for exec-time calibration
            for _rep in range(repeat):
                qT_sb = persist.tile(
                    [P, QK // P, QS], F32R, tag="qT", name="qT_sb"
                )
                kT_sb = persist.tile(
                    [P, QK // P, KVLEN], F32R, tag="kT", name="kT_sb"
                )
                v_sb = persist.tile(
                    [P, KVLEN // P, H, VH + 1], BF16, tag="v", name="v_sb"
                )
                rT_sb = persist.tile(
                    [P, VC // P, QS], BF16, tag="rT", name="rT_sb"
                )
                for k in range(KVLEN // P):
                    nc.vector.tensor_copy(v_sb[:, k, :, VH], onesf[:])

                # ---- Q projection (own SBUF/PSUM scope, freed afterwards)
                with (
                    tc.tile_pool(name="xqp", bufs=1) as xqp,
                    tc.tile_pool(name="qpp", bufs=4, space="PSUM") as qpp,
                ):
                    xqT_sb = xqp.tile([P, KD, QS], F32R, name="xqT_sb")
                    wq = []
                    for k in range(KD):
                        nc.sync.dma_start(out=xqT_sb[:, k, :], in_=xqT_d[k])
                        wt = wpool.tile([P, 1024], F32R, tag="w", name="wt")
                        nc.sync.dma_start(out=wt[:], in_=WqT_d[k])
                        wq.append(wt)
                    nc.sync.dma_start(out=bias_sb[:], in_=bias_d[:])
                    for mh in range(2):
                        qps = [
                            qpp.tile([P, QS], F32, tag="qpp", name="qps")
                            for _ in range(4)
                        ]
                        for k in range(KD):
                            for mi in range(4):
                                m = mh * 4 + mi
                                nc.tensor.matmul(
                                    qps[mi][:],
                                    wq[k][:, m * P:(m + 1) * P],
                                    xqT_sb[:, k, :],
                                    start=(k == 0),
                                    stop=(k == KD - 1),
                                )
                        for mi in range(4):
                            m = mh * 4 + mi
                            nc.vector.tensor_scalar_add(
                                qT_sb[:, m, :], qps[mi][:], bq_sb[:, m:m + 1]
                            )

                # ---- K/V projections + attention heads (interleaved)
                with (
                    tc.tile_pool(name="xkvp", bufs=1) as xkvp,
                    tc.tile_pool(name="eTp", bufs=6) as eTp,
                    tc.tile_pool(name="pTp", bufs=4) as pTp,
                    tc.tile_pool(name="bcrp", bufs=2) as bcrp,
                    tc.tile_pool(name="sump", bufs=2) as sump,
                    tc.tile_pool(name="brp", bufs=1) as brp,
                    tc.tile_pool(name="pp", bufs=2, space="PSUM") as pp,
                    tc.tile_pool(name="psc", bufs=3, space="PSUM") as psc,
                    tc.tile_pool(name="pres", bufs=2, space="PSUM") as pres,
                    tc.tile_pool(name="pbc", bufs=1, space="PSUM") as pbc,
                ):
                    xkvT_sb = xkvp.tile([P, KD, KVLEN], F32R, name="xkvT_sb")
                    wk = []
                    for k in range(KD):
                        nc.sync.dma_start(out=xkvT_sb[:, k, :], in_=xkvT_d[k])
                        wt = wpool.tile([P, 1024], F32R, tag="w", name="wt")
                        nc.sync.dma_start(out=wt[:], in_=WkT_d[k])
                        wk.append(wt)

                    # K projection: k-outer (kv half, m quarter) passes,
                    # only 2 PSUM banks held
                    for n in range(2):
                        for mh in range(4):
                            kps = [
                                pp.tile([P, QS], F32, tag="pp", name="kps")
                                for _ in range(2)
                            ]
                            for k in range(KD):
                                for mi in range(2):
                                    m = mh * 2 + mi
                                    nc.tensor.matmul(
                                        kps[mi][:],
                                        wk[k][:, m * P:(m + 1) * P],
                                        xkvT_sb[:, k, n * QS:(n + 1) * QS],
                                        start=(k == 0),
                                        stop=(k == KD - 1),
                                    )
                            for mi in range(2):
                                m = mh * 2 + mi
                                nc.vector.tensor_scalar_add(
                                    kT_sb[:, m, n * QS:(n + 1) * QS],
                                    kps[mi][:], bk_sb[:, m:m + 1]
                                )

                    wv = []
                    for k in range(KD):
                        wt = wpool.tile([P, 1024], F32R, tag="w", name="wt")
                        nc.sync.dma_start(out=wt[:], in_=WvT_d[k])
                        wv.append(wt)

                    def v_quarter(n, mq4):
                        """One (vc-half, m-quarter) k-outer V-projection pass
                        holding only 2 PSUM banks."""
                        vps = [
                            pp.tile([P, QS], F32, tag="pp", name="vps")
                            for _ in range(2)
                        ]
                        for k in range(KD):
                            for mi in range(2):
                                m = mq4 * 2 + mi
                                nc.tensor.matmul(
                                    vps[mi][:],
                                    xkvT_sb[:, k, m * P:(m + 1) * P],
                                    wv[k][:, n * QS:(n + 1) * QS],
                                    start=(k == 0),
                                    stop=(k == KD - 1),
                                )
                        for mi in range(2):
                            m = mq4 * 2 + mi
                            nc.scalar.copy(
                                v_sb[:, m, 8 * n:8 * n + 8, 0:VH],
                                vps[mi][:].rearrange("p (h c) -> p h c", h=8),
                            )

                    def probs_part(h, eT):
                        """scores/exp already done; softmax sums via a
                        ones-column matmul (no V dependency), normalize and
                        DMA the probs out."""
                        bc = pbc.tile([P, QS], F32, tag="bc", name="bc")
                        for k in range(KVLEN // P):
                            nc.tensor.matmul(
                                bc[0:1, :],
                                ones_col_bf[:],
                                eT[:, k, :],
                                start=(k == 0),
                                stop=(k == KVLEN // P - 1),
                            )
                        srt = sump.tile([1, QS], F32R, tag="sums", name="srt")
                        nc.vector.tensor_copy(srt[:], bc[0:1, :])
                        nc.tensor.matmul(
                            bc[:], ones1_r[:], srt[:], start=True, stop=True
                        )
                        bcr = bcrp.tile([P, QS], F32, tag="bcr", name="bcr")
                        nc.vector.reciprocal(bcr[:], bc[:])
                        for jg in range(4):
                            pT = pTp.tile([P, 2, QS], BF16, tag="pT", name="pT")
                            for jj in range(2):
                                j = jg * 2 + jj
                                eng = nc.vector if jj == 0 else nc.gpsimd
                                eng.tensor_mul(
                                    pT[:, jj, :], eT[:, j, :], bcr[:]
                                )
                            nc.sync.dma_start(
                                out=probsT_d[h, jg * 2:(jg + 1) * 2].rearrange(
                                    "j p q -> p j q"
                                ),
                                in_=pT[:],
                            )

                    def result_part(h, eT):
                        """attention result for one head; the reciprocal is
                        recomputed from the v-aug sums row so nothing from
                        probs_part needs to stay alive."""
                        mq = h >> 1
                        bp = 64 * (h & 1)
                        rp = pres.tile([VH + 1, QS], F32, tag="res", name="rp")
                        for k in range(KVLEN // P):
                            nc.tensor.matmul(
                                rp[:],
                                v_sb[:, k, h, :],
                                eT[:, k, :],
                                start=(k == 0),
                                stop=(k == KVLEN // P - 1),
                            )
                        srt = sump.tile([1, QS], F32R, tag="sums", name="srt")
                        nc.vector.tensor_copy(srt[:], rp[VH:VH + 1, :])
                        bc2 = pbc.tile([P, QS], F32, tag="bc", name="bc2")
                        nc.tensor.matmul(
                            bc2[0:VH, :], ones1_r[:, 0:VH], srt[:],
                            start=True, stop=True,
                        )
                        br = brp.tile([VH, QS], F32, tag="br", name="br")
                        nc.vector.reciprocal(br[:], bc2[0:VH, :])
                        return nc.vector.tensor_mul(
                            rT_sb[bp:bp + 64, mq, :], rp[0:VH, :], br[:]
                        )

                    def full_head(h, eT):
                        """Fused head: result matmul first (v ready), softmax
                        sums from the v-aug row, then probs + resultT."""
                        mq = h >> 1
                        bp = 64 * (h & 1)
                        rp = pres.tile([VH + 1, QS], F32, tag="res", name="rp")
                        for k in range(KVLEN // P):
                            nc.tensor.matmul(
                                rp[:],
                                v_sb[:, k, h, :],
                                eT[:, k, :],
                                start=(k == 0),
                                stop=(k == KVLEN // P - 1),
                            )
                        srt = sump.tile([1, QS], F32R, tag="sums", name="srt")
                        nc.vector.tensor_copy(srt[:], rp[VH:VH + 1, :])
                        bc = pbc.tile([P, QS], F32, tag="bc", name="bc")
                        nc.tensor.matmul(
                            bc[:], ones1_r[:], srt[:], start=True, stop=True
                        )
                        bcr = bcrp.tile([P, QS], F32, tag="bcr", name="bcr")
                        nc.vector.reciprocal(bcr[:], bc[:])
                        for jg in range(4):
                            pT = pTp.tile([P, 2, QS], BF16, tag="pT", name="pT")
                            for jj in range(2):
                                j = jg * 2 + jj
                                eng = nc.vector if jj == 0 else nc.gpsimd
                                eng.tensor_mul(
                                    pT[:, jj, :], eT[:, j, :], bcr[:]
                                )
                            nc.sync.dma_start(
                                out=probsT_d[h, jg * 2:(jg + 1) * 2].rearrange(
                                    "j p q -> p j q"
                                ),
                                in_=pT[:],
                            )
                        nc.vector.tensor_mul(
                            rT_sb[bp:bp + 64, mq, :], rp[0:VH, :], bcr[0:64, :]
                        )

                    def pair_scores(i, after=None):
                        """scores+exp for heads (2i, 2i+1), j-interleaved so
                        the K=64 row-packed pairs overlap on the PE array.
                        `after` pins an ordering edge so the scheduler keeps
                        this behind the result whose expT slots we reuse."""
                        h0, h1 = 2 * i, 2 * i + 1
                        mq = i
                        eT0 = eTp.tile(
                            [P, KVLEN // P, QS], BF16, tag="eT", name="eT0"
                        )
                        eT1 = eTp.tile(
                            [P, KVLEN // P, QS], BF16, tag="eT", name="eT1"
                        )
                        eTs = (eT0, eT1)
                        for j in range(KVLEN // P):
                            for x, hh in enumerate((h0, h1)):
                                bp = 64 * (hh & 1)
                                sc = psc.tile(
                                    [P, QS], F32, tag="sc", name="sc"
                                )
                                mm = nc.tensor.matmul(
                                    sc[:],
                                    kT_sb[bp:bp + 64, mq, j * P:(j + 1) * P],
                                    qT_sb[bp:bp + 64, mq, :],
                                    start=True,
                                    stop=True,
                                )
                                if after is not None:
                                    add_dep_helper(
                                        mm.ins, after.ins, sync=False,
                                        reason="expT slot pipeline order",
                                    )
                                    after = None
                                nc.scalar.activation(
                                    eTs[x][:, j, :], sc[:], Exp, scale=0.125
                                )
                        return eTs

                    # Schedule: pairs 0,1,4,5 run probs-first with
                    # ones-column sums (no V dependency) interleaved with the
                    # V quarter passes; their results follow once the V half
                    # is ready. Pairs 2,3,6,7 run fused (V already there).
                    eTs = {}

                    def S(i):
                        eTs[i] = pair_scores(i)

                    def Pp(i):
                        probs_part(2 * i, eTs[i][0])
                        probs_part(2 * i + 1, eTs[i][1])

                    def R(i):
                        result_part(2 * i, eTs[i][0])
                        result_part(2 * i + 1, eTs[i][1])
                        del eTs[i]

                    def F(i):
                        full_head(2 * i, eTs[i][0])
                        full_head(2 * i + 1, eTs[i][1])
                        del eTs[i]

                    S(0); Pp(0)
                    v_quarter(0, 0)
                    S(1); Pp(1)
                    v_quarter(0, 1); v_quarter(0, 2); v_quarter(0, 3)
                    R(0); S(2); F(2)
                    R(1); S(3); F(3)
                    v_quarter(1, 0); S(4); Pp(4)
                    v_quarter(1, 1); S(5); Pp(5)
                    v_quarter(1, 2); v_quarter(1, 3)
                    wo = []
                    for k in range(VC // P):
                        wt = wpool.tile([P, 1024], BF16, tag="w", name="wt")
                        nc.sync.dma_start(out=wt[:], in_=WoT_d[k])
                        wo.append(wt)
                    R(4); S(6); F(6)
                    R(5); S(7); F(7)

                # ---- output projection
                with (
                    tc.tile_pool(name="otp", bufs=2) as otp,
                    tc.tile_pool(name="ppo", bufs=4, space="PSUM") as ppo,
                ):
                    # k-outer over m-halves: chunks consumed in
                    # head-completion order so most of the projection
                    # overlaps the last heads
                    for mh in range(2):
                        pos = [
                            ppo.tile([P, QS], F32, tag="po", name="po")
                            for _ in range(4)
                        ]
                        for k in range(VC // P):
                            for mi in range(4):
                                m = mh * 4 + mi
                                nc.tensor.matmul(
                                    pos[mi][:],
                                    wo[k][:, m * P:(m + 1) * P],
                                    rT_sb[:, k, :],
                                    start=(k == 0),
                                    stop=(k == VC // P - 1),
                                )
                        for mi in range(4):
                            m = mh * 4 + mi
                            ot = otp.tile([P, QS], BF16, tag="ot", name="ot")
                            nc.vector.tensor_scalar_add(
                                ot[:], pos[mi][:], bco_sb[:, m:m + 1]
                            )
                            nc.sync.dma_start(out=outT_d[m], in_=ot[:])

    nc.compile()
    return nc


def get_nc(repeat: int = 1):
    if repeat not in _NC_CACHE_R:
        _NC_CACHE_R[repeat] = _build(repeat)
    return _NC_CACHE_R[repeat]


def make_in_maps(inputs_q, inputs_kv, Wq, bq, Wk, bk, Wv, bv, Wo, bo):
    inputs_q = np.asarray(inputs_q, dtype=np.float32)
    inputs_kv = np.asarray(inputs_kv, dtype=np.float32)
    Wq = np.asarray(Wq, dtype=np.float32)
    Wk = np.asarray(Wk, dtype=np.float32)
    Wv = np.asarray(Wv, dtype=np.float32)
    Wo = np.asarray(Wo, dtype=np.float32)
    bq = np.asarray(bq, dtype=np.float32)
    bk = np.asarray(bk, dtype=np.float32)
    bv = np.asarray(bv, dtype=np.float32)
    bo = np.asarray(bo, dtype=np.float32)

    WqT = round_f32r(Wq.T).reshape(KD, P, QK)
    WkT = round_f32r(Wk.T).reshape(KD, P, QK)
    WvT = round_f32r(Wv.T).reshape(KD, P, VC)
    import ml_dtypes
    WoT = np.ascontiguousarray(Wo.T).astype(ml_dtypes.bfloat16).reshape(VC // P, P, OC)
    bco = (bo + Wo @ bv).astype(np.float32)
    bias_all = np.concatenate([
        bq.reshape(8, P).T, bk.reshape(8, P).T, bco.reshape(8, P).T
    ], axis=1)  # [P, 24]

    in_maps = []
    for c in range(NCORES):
        bg, half = divmod(c, 2)
        b, g = divmod(bg, G)
        xq = inputs_q[b, g, half * QS:(half + 1) * QS, :]       # [QS, DIN]
        xkv = inputs_kv[b, g]                                   # [KVLEN, DIN]
        xqT = round_f32r(np.ascontiguousarray(xq.T)).reshape(KD, P, QS)
        xkvT = round_f32r(np.ascontiguousarray(xkv.T)).reshape(KD, P, KVLEN)
        in_maps.append({
            "xqT": xqT, "xkvT": xkvT,
            "WqT": WqT, "WkT": WkT, "WvT": WvT, "WoT": WoT,
            "bias_all": bias_all,
        })
    return in_maps


def assemble(results):
    out = np.empty((B, G, QLEN, OC), np.float32)
    probs = np.empty((B, G, H, QLEN, KVLEN), np.float32)
    for c in range(NCORES):
        bg, half = divmod(c, 2)
        b, g = divmod(bg, G)
        qs = slice(half * QS, (half + 1) * QS)
        outT = np.asarray(
            results[c]["outT"], dtype=np.float32
        ).reshape(OC, QS)                                        # [oc, q]
        out[b, g, qs, :] = outT.T
        pT = np.asarray(
            results[c]["probsT"], dtype=np.float32
        ).reshape(H, KVLEN, QS)                                  # [h, kv, q]
        probs[b, g, :, qs, :] = pT.transpose(0, 2, 1)
    return out, probs


def kernel(inputs_q, inputs_kv, Wq, bq, Wk, bk, Wv, bv, Wo, bo):
    nc = get_nc()
    in_maps = make_in_maps(
        inputs_q, inputs_kv, Wq, bq, Wk, bk, Wv, bv, Wo, bo
    )
    res = run_bass_kernel_spmd(nc, in_maps, core_ids=list(range(NCORES)))
    return assemble(res.results)
